# revision 7
# baseline (speedup 1.0000x reference)
"""Bass/Tile Trainium2 kernel for nn_CrossAttentionLayer.

Reference computation (per batch b):
    Q = h1 @ Wq.T; K = h2 @ Wk.T; V = h2 @ Wv.T
    E = Q @ K.T;  E = where(mask==0, -1e10, E)
    A = softmax(E / sqrt(HID), axis=-1)
    out = A @ V

Strategy:
  - Data-parallel over batch: 8 batches -> 8 NeuronCores (SPMD, one NEFF).
  - Algebraic fusion: E = Q K^T = h1 (Wq^T Wk) h2^T = h1 G h2^T with
    G = Wq^T @ Wk precomputed on host (tiny 1024^3 matmul). This removes one
    full [N,D]x[D,HID] projection from the device.
  - "Transposed scores" dataflow: compute E^T tiles [m(part), n(free)] so the
    A@V matmul can consume the probabilities directly as the stationary
    operand (contraction over m = partition dim), no on-chip transpose of A.
  - Softmax: logits E/32 ~ N(0,1) so exp() needs no max-subtraction; masked
    entries are exactly zeroed by multiplying with the (0/1) mask after exp,
    which matches the reference's -1e10 masking bit-for-bit in spirit
    (exp(-1e10/32 - max) underflows to 0 in fp32).
  - Softmax denominators come for free from an extra 1-column matmul
    (P^T @ ones) sharing the stationary operand with the A@V matmuls; the
    1/denom scaling is folded into the PSUM->SBUF output eviction.
  - All big transposes (h1^T, h2^T, mask^T) ride the DMA xbar transpose
    during the HBM->SBUF load (bf16), costing zero PE/DVE/ACT time.
  - bf16 matmuls (PE full rate), fp32 PSUM accumulation, fp32 output.
"""

import math
import sys

import numpy as np

sys.path.insert(0, "/opt/trn_rl_repo")

import ml_dtypes

import concourse.bass as bass
import concourse.tile as tile
from concourse import bacc, mybir
from concourse.bass_utils import run_bass_kernel_spmd

BF16 = mybir.dt.bfloat16
F32 = mybir.dt.float32

# Problem dims (hardcoded per harness contract).
B, N, M, D, HID, OUT = 8, 2048, 2048, 1024, 1024, 1024
N_CORES = 8
P = 128


def emit_kernel(tc, h1, h2, maskf, G, WvT, ones, out, n, m, d, o, free):
    """Emit the per-core attention program.

    h1:    DRAM [n, d]   bf16   (this core's batch of h1)
    h2:    DRAM [m, d]   bf16
    maskf: DRAM [n, m]   bf16   (0.0 / 1.0)
    G:     DRAM [d, d]   bf16   (Wq^T @ Wk)
    WvT:   DRAM [d, o]   bf16   (Wv^T)
    ones:  DRAM [P, 1]   bf16
    out:   DRAM [n, o]   f32
    """
    nc = tc.nc
    KC = d // P  # contraction chunks along d
    MC = m // P  # m chunks (score partition dim)
    NB = n // free  # n macro blocks
    NS = free // P  # n sub-chunks per block (output partition dim)
    OB = o // free  # output free-dim blocks
    rscale = 1.0 / math.sqrt(HID)

    with tc.tile_pool(name="persist", bufs=1) as persist:
        # ---- persistent SBUF tensors for phase B
        h2T = persist.tile([P, KC, m], BF16)  # h2^T  [d(part), m]
        QGT = persist.tile([P, KC, n], BF16)  # (h1 G)^T  [d'(part), n]
        V = persist.tile([P, MC, o], BF16)  # V  [m(part), o]
        ones_sb = persist.tile([P, 1], BF16)
        nc.sync.dma_start(ones_sb[:], ones[:])

        # ---- phase A: transposed loads + projections ----
        with (
            tc.tile_pool(name="phaseA", bufs=1) as pA,
            tc.tile_pool(name="psA", bufs=4, space="PSUM") as psA,
        ):
            G_sb = pA.tile([P, KC, d], BF16)
            WvT_sb = pA.tile([P, KC, o], BF16)
            h1T = pA.tile([P, KC, n], BF16)
            nc.sync.dma_start(G_sb[:], G.rearrange("(kc p) e -> p kc e", p=P))
            nc.sync.dma_start(WvT_sb[:], WvT.rearrange("(kc p) e -> p kc e", p=P))
            for kc in range(KC):
                nc.sync.dma_start(
                    h1T[:, kc, :], h1[:, kc * P : (kc + 1) * P], transpose=True
                )
                nc.sync.dma_start(
                    h2T[:, kc, :], h2[:, kc * P : (kc + 1) * P], transpose=True
                )

            # QGT[d',nb] = sum_dc G[dc, d']^T . h1T[dc, nb]
            for dc2 in range(KC):
                for nb in range(n // free):
                    ps = psA.tile([P, free], F32)
                    for dc in range(KC):
                        nc.tensor.matmul(
                            ps[:],
                            lhsT=G_sb[:, dc, dc2 * P : (dc2 + 1) * P],
                            rhs=h1T[:, dc, nb * free : (nb + 1) * free],
                            start=(dc == 0),
                            stop=(dc == KC - 1),
                        )
                    nc.scalar.copy(QGT[:, dc2, nb * free : (nb + 1) * free], ps[:])

            # V[mc, ob] = sum_dc h2T[dc, mc]^T . WvT[dc, ob]
            for mc in range(MC):
                for ob in range(OB):
                    ps = psA.tile([P, free], F32)
                    for dc in range(KC):
                        nc.tensor.matmul(
                            ps[:],
                            lhsT=h2T[:, dc, mc * P : (mc + 1) * P],
                            rhs=WvT_sb[:, dc, ob * free : (ob + 1) * free],
                            start=(dc == 0),
                            stop=(dc == KC - 1),
                        )
                    nc.scalar.copy(V[:, mc, ob * free : (ob + 1) * free], ps[:])

        # ---- phase B: scores^T -> exp -> mask -> A^T V ----
        with (
            tc.tile_pool(name="etpsum", bufs=2, space="PSUM") as etpsum,
            tc.tile_pool(name="avpsum", bufs=2, space="PSUM") as avpsum,
            tc.tile_pool(name="denpsum", bufs=2, space="PSUM") as denpsum,
            tc.tile_pool(name="maskp", bufs=2) as maskp,
            tc.tile_pool(name="ptp", bufs=2) as ptp,
            tc.tile_pool(name="outp", bufs=3) as outp,
            tc.tile_pool(name="smalls", bufs=4) as smalls,
        ):
            for nb in range(NB):
                nsl = slice(nb * free, (nb + 1) * free)
                # mask^T panel for this n block (transposed load via xbar)
                mT = maskp.tile([P, MC, free], BF16)
                for mc in range(MC):
                    nc.sync.dma_start(
                        mT[:, mc, :],
                        maskf[nsl, mc * P : (mc + 1) * P],
                        transpose=True,
                    )

                # P^T tiles: PT[m(part), n(free)] = exp(E^T/32) * mask^T
                PT = ptp.tile([P, MC, free], BF16)
                for mc in range(MC):
                    ps = etpsum.tile([P, free], F32)
                    for dc in range(KC):
                        nc.tensor.matmul(
                            ps[:],
                            lhsT=h2T[:, dc, mc * P : (mc + 1) * P],
                            rhs=QGT[:, dc, nsl],
                            start=(dc == 0),
                            stop=(dc == KC - 1),
                        )
                    nc.scalar.activation(
                        PT[:, mc, :], ps[:], mybir.ActivationFunctionType.Exp,
                        scale=rscale,
                    )
                    nc.vector.tensor_mul(PT[:, mc, :], PT[:, mc, :], mT[:, mc, :])

                # out[ns] = (PT[:, ns]^T @ V) / (PT[:, ns]^T @ 1)
                for ns in range(NS):
                    po = [
                        avpsum.tile([P, free], F32, name=f"po{ob}", tag=f"po{ob}")
                        for ob in range(OB)
                    ]
                    pden = denpsum.tile([P, 1], F32)
                    for mc in range(MC):
                        lhs = PT[:, mc, ns * P : (ns + 1) * P]
                        for ob in range(OB):
                            nc.tensor.matmul(
                                po[ob][:],
                                lhsT=lhs,
                                rhs=V[:, mc, ob * free : (ob + 1) * free],
                                start=(mc == 0),
                                stop=(mc == MC - 1),
                            )
                        nc.tensor.matmul(
                            pden[:],
                            lhsT=lhs,
                            rhs=ones_sb[:],
                            start=(mc == 0),
                            stop=(mc == MC - 1),
                        )
                    rden = smalls.tile([P, 1], F32)
                    nc.vector.reciprocal(rden[:], pden[:])
                    ob_sb = outp.tile([P, o], F32)
                    for ob in range(OB):
                        nc.scalar.activation(
                            ob_sb[:, ob * free : (ob + 1) * free],
                            po[ob][:],
                            mybir.ActivationFunctionType.Copy,
                            scale=rden[:],
                        )
                    r0 = nb * free + ns * P
                    nc.sync.dma_start(out[r0 : r0 + P, :], ob_sb[:])


def build_nc(n=N, m=M, d=D, o=OUT, n_cores=N_CORES, free=512):
    nc = bacc.Bacc(
        "TRN2",
        target_bir_lowering=False,
        debug=False,
        enable_asserts=False,
        num_devices=n_cores,
    )
    h1 = nc.dram_tensor("h1", [n, d], BF16, kind="ExternalInput").ap()
    h2 = nc.dram_tensor("h2", [m, d], BF16, kind="ExternalInput").ap()
    maskf = nc.dram_tensor("maskf", [n, m], BF16, kind="ExternalInput").ap()
    G = nc.dram_tensor("G", [d, d], BF16, kind="ExternalInput").ap()
    WvT = nc.dram_tensor("WvT", [d, o], BF16, kind="ExternalInput").ap()
    ones = nc.dram_tensor("ones", [P, 1], BF16, kind="ExternalInput").ap()
    out = nc.dram_tensor("out", [n, o], F32, kind="ExternalOutput").ap()
    with tile.TileContext(nc) as tc:
        emit_kernel(tc, h1, h2, maskf, G, WvT, ones, out, n, m, d, o, free)
    nc.compile()
    return nc


def _to_bf16(x_f32):
    """Fast vectorized fp32 -> bf16 with round-to-nearest-even."""
    x = np.ascontiguousarray(x_f32, dtype=np.float32)
    u = x.view(np.uint32)
    r = ((u >> np.uint32(16)) & np.uint32(1)) + np.uint32(0x7FFF)
    return ((u + r) >> np.uint32(16)).astype(np.uint16).view(ml_dtypes.bfloat16)


def prep_inputs(h1, h2, mask, Wq, Wk, Wv):
    """Host-side prep: fold Wq/Wk into G, transpose Wv, bf16-convert."""
    G = _to_bf16(Wq.astype(np.float32, copy=False).T @ Wk.astype(np.float32, copy=False))
    WvT = _to_bf16(np.ascontiguousarray(Wv.astype(np.float32, copy=False).T))
    h1b = _to_bf16(h1)
    h2b = _to_bf16(h2)
    # mask is 0/1 int32 -> bf16 0.0/1.0 via integer trick (0x3F80 == bf16 1.0)
    mb = (mask.astype(np.uint16) * np.uint16(0x3F80)).view(ml_dtypes.bfloat16)
    ones = np.ones((P, 1), dtype=ml_dtypes.bfloat16)
    return [
        {
            "h1": h1b[b],
            "h2": h2b[b],
            "maskf": mb[b],
            "G": G,
            "WvT": WvT,
            "ones": ones,
        }
        for b in range(B)
    ]


_NC_CACHE = {}


def get_nc():
    if "nc" not in _NC_CACHE:
        _NC_CACHE["nc"] = build_nc()
    return _NC_CACHE["nc"]


def run(in_maps, trace=False):
    return run_bass_kernel_spmd(get_nc(), in_maps, list(range(N_CORES)), trace=trace)


def kernel(h1, h2, mask, Wq, Wk, Wv):
    in_maps = prep_inputs(h1, h2, mask, Wq, Wk, Wv)
    res = run(in_maps)
    return np.stack([res.results[b]["out"] for b in range(B)], axis=0)


# revision 13
# speedup vs baseline: 147.0798x; 147.0798x over previous
"""Bass/Tile Trainium2 kernel for nn_CrossAttentionLayer.

Reference computation (per batch b):
    Q = h1 @ Wq.T; K = h2 @ Wk.T; V = h2 @ Wv.T
    E = Q @ K.T;  E = where(mask==0, -1e10, E)
    A = softmax(E / sqrt(HID), axis=-1)
    out = A @ V

Strategy:
  - Data-parallel over batch: 8 batches -> 8 NeuronCores (SPMD, one NEFF).
  - Algebraic fusion: E = Q K^T = h1 (Wq^T Wk) h2^T = h1 G h2^T with
    G = Wq^T @ Wk precomputed on host (tiny 1024^3 matmul). This removes one
    full [N,D]x[D,HID] projection from the device.
  - "Transposed scores" dataflow: compute E^T tiles [m(part), n(free)] so the
    A@V matmul can consume the probabilities directly as the stationary
    operand (contraction over m = partition dim), no on-chip transpose of A.
  - Softmax: logits E/32 ~ N(0,1) so exp() needs no max-subtraction; masked
    entries are exactly zeroed by multiplying with the (0/1) mask after exp,
    which matches the reference's -1e10 masking bit-for-bit in spirit
    (exp(-1e10/32 - max) underflows to 0 in fp32).
  - Softmax denominators come for free from an extra 1-column matmul
    (P^T @ ones) sharing the stationary operand with the A@V matmuls; the
    1/denom scaling is folded into the PSUM->SBUF output eviction.
  - All big transposes (h1^T, h2^T, mask^T) ride the DMA xbar transpose
    during the HBM->SBUF load (bf16), costing zero PE/DVE/ACT time.
  - bf16 matmuls (PE full rate), fp32 PSUM accumulation, fp32 output.
"""

import math
import sys

import numpy as np

sys.path.insert(0, "/opt/trn_rl_repo")

import ml_dtypes

import concourse.bass as bass
import concourse.tile as tile
from concourse import bacc, mybir
from concourse.bass_utils import run_bass_kernel_spmd

BF16 = mybir.dt.bfloat16
F32 = mybir.dt.float32

# Problem dims (hardcoded per harness contract).
B, N, M, D, HID, OUT = 8, 2048, 2048, 1024, 1024, 1024
N_CORES = 8
P = 128


def emit_kernel(tc, h1, h2, maskf, G, WvT, ones, out, n, m, d, o, free):
    """Emit the per-core attention program.

    h1:    DRAM [n, d]   bf16   (this core's batch of h1)
    h2:    DRAM [m, d]   bf16
    maskf: DRAM [n, m]   bf16   (0.0 / 1.0)
    G:     DRAM [d, d]   bf16   (Wq^T @ Wk)
    WvT:   DRAM [d, o]   bf16   (Wv^T)
    ones:  DRAM [P, 1]   bf16
    out:   DRAM [n, o]   f32
    """
    nc = tc.nc
    KC = d // P  # contraction chunks along d
    MC = m // P  # m chunks (score partition dim)
    NB = n // free  # n macro blocks
    NS = free // P  # n sub-chunks per block (output partition dim)
    OB = o // free  # output free-dim blocks
    rscale = 1.0 / math.sqrt(HID)

    with tc.tile_pool(name="persist", bufs=1) as persist:
        # ---- persistent SBUF tensors for phase B
        h2T = persist.tile([P, KC, m], BF16)  # h2^T  [d(part), m]
        QGT = persist.tile([P, KC, n], BF16)  # (h1 G)^T  [d'(part), n]
        V = persist.tile([P, MC, o], BF16)  # V  [m(part), o]
        ones_sb = persist.tile([P, 1], BF16)
        nc.sync.dma_start(ones_sb[:], ones[:])

        # ---- phase A: transposed loads + projections ----
        with tc.tile_pool(name="phaseA", bufs=1) as pA:
            G_sb = pA.tile([P, KC, d], BF16)
            WvT_sb = pA.tile([P, KC, o], BF16)
            h1T = pA.tile([P, KC, n], BF16)
            nc.sync.dma_start(G_sb[:], G.rearrange("(kc p) e -> p kc e", p=P))
            nc.sync.dma_start(WvT_sb[:], WvT.rearrange("(kc p) e -> p kc e", p=P))
            # Transposed loads in (kc x 512-col) pieces so the first matmuls
            # can start as soon as the first column block lands.
            for nb in range(n // free):
                for kc in range(KC):
                    nsl = slice(nb * free, (nb + 1) * free)
                    nc.sync.dma_start(
                        h1T[:, kc, nsl], h1[nsl, kc * P : (kc + 1) * P], transpose=True
                    )
            for mb in range(m // free):
                for kc in range(KC):
                    msl = slice(mb * free, (mb + 1) * free)
                    nc.sync.dma_start(
                        h2T[:, kc, msl], h2[msl, kc * P : (kc + 1) * P], transpose=True
                    )

            # QGT[d',nb] = sum_dc G[dc, d']^T . h1T[dc, nb]
            # dc innermost-but-one, nb innermost: 4 consecutive matmuls share
            # the stationary G[dc, dc2] block -> 1 weight load per (dc2, dc).
            NBB = n // free
            with tc.tile_pool(name="psQ", bufs=2, space="PSUM") as psQ:
                for dc2 in range(KC):
                    ps_nb = [
                        psQ.tile([P, free], F32, name=f"ps{nb}", tag=f"ps{nb}")
                        for nb in range(NBB)
                    ]
                    for dc in range(KC):
                        for nb in range(NBB):
                            nc.tensor.matmul(
                                ps_nb[nb][:],
                                lhsT=G_sb[:, dc, dc2 * P : (dc2 + 1) * P],
                                rhs=h1T[:, dc, nb * free : (nb + 1) * free],
                                start=(dc == 0),
                                stop=(dc == KC - 1),
                            )
                    for nb in range(NBB):
                        nc.scalar.copy(
                            QGT[:, dc2, nb * free : (nb + 1) * free], ps_nb[nb][:]
                        )

            # V[mc, ob] = sum_dc h2T[dc, mc]^T . WvT[dc, ob]
            # ob innermost: OB consecutive matmuls share h2T[dc, mc].
            with tc.tile_pool(name="psV", bufs=2, space="PSUM") as psV:
                for mc in range(MC):
                    ps_ob = [
                        psV.tile([P, free], F32, name=f"psv{ob}", tag=f"psv{ob}")
                        for ob in range(OB)
                    ]
                    for dc in range(KC):
                        for ob in range(OB):
                            nc.tensor.matmul(
                                ps_ob[ob][:],
                                lhsT=h2T[:, dc, mc * P : (mc + 1) * P],
                                rhs=WvT_sb[:, dc, ob * free : (ob + 1) * free],
                                start=(dc == 0),
                                stop=(dc == KC - 1),
                            )
                    for ob in range(OB):
                        nc.scalar.copy(
                            V[:, mc, ob * free : (ob + 1) * free], ps_ob[ob][:]
                        )

        # ---- phase B: scores^T -> exp -> mask -> A^T V ----
        with (
            tc.tile_pool(name="etpsum", bufs=2, space="PSUM") as etpsum,
            tc.tile_pool(name="avpsum", bufs=2, space="PSUM") as avpsum,
            tc.tile_pool(name="denpsum", bufs=2, space="PSUM") as denpsum,
            tc.tile_pool(name="maskp", bufs=2) as maskp,
            tc.tile_pool(name="ptp", bufs=2) as ptp,
            tc.tile_pool(name="outp", bufs=3) as outp,
            tc.tile_pool(name="smalls", bufs=4) as smalls,
        ):
            for nb in range(NB):
                nsl = slice(nb * free, (nb + 1) * free)
                # mask^T panel for this n block (transposed load via xbar)
                mT = maskp.tile([P, MC, free], BF16)
                for mc in range(MC):
                    nc.sync.dma_start(
                        mT[:, mc, :],
                        maskf[nsl, mc * P : (mc + 1) * P],
                        transpose=True,
                    )

                # P^T tiles: PT[m(part), n(free)] = exp(E^T/32) * mask^T
                PT = ptp.tile([P, MC, free], BF16)
                for mc in range(MC):
                    ps = etpsum.tile([P, free], F32)
                    for dc in range(KC):
                        nc.tensor.matmul(
                            ps[:],
                            lhsT=h2T[:, dc, mc * P : (mc + 1) * P],
                            rhs=QGT[:, dc, nsl],
                            start=(dc == 0),
                            stop=(dc == KC - 1),
                        )
                    nc.scalar.activation(
                        PT[:, mc, :], ps[:], mybir.ActivationFunctionType.Exp,
                        scale=rscale,
                    )
                    nc.vector.tensor_mul(PT[:, mc, :], PT[:, mc, :], mT[:, mc, :])

                # out[ns] = (PT[:, ns]^T @ V) / (PT[:, ns]^T @ 1)
                for ns in range(NS):
                    po = [
                        avpsum.tile([P, free], F32, name=f"po{ob}", tag=f"po{ob}")
                        for ob in range(OB)
                    ]
                    pden = denpsum.tile([P, 1], F32)
                    for mc in range(MC):
                        lhs = PT[:, mc, ns * P : (ns + 1) * P]
                        for ob in range(OB):
                            nc.tensor.matmul(
                                po[ob][:],
                                lhsT=lhs,
                                rhs=V[:, mc, ob * free : (ob + 1) * free],
                                start=(mc == 0),
                                stop=(mc == MC - 1),
                            )
                        nc.tensor.matmul(
                            pden[:],
                            lhsT=lhs,
                            rhs=ones_sb[:],
                            start=(mc == 0),
                            stop=(mc == MC - 1),
                        )
                    rden = smalls.tile([P, 1], F32)
                    nc.vector.reciprocal(rden[:], pden[:])
                    ob_sb = outp.tile([P, o], F32)
                    for ob in range(OB):
                        nc.scalar.activation(
                            ob_sb[:, ob * free : (ob + 1) * free],
                            po[ob][:],
                            mybir.ActivationFunctionType.Copy,
                            scale=rden[:],
                        )
                    r0 = nb * free + ns * P
                    nc.sync.dma_start(out[r0 : r0 + P, :], ob_sb[:])


def build_nc(n=N, m=M, d=D, o=OUT, n_cores=N_CORES, free=512, reps=1):
    nc = bacc.Bacc(
        "TRN2",
        target_bir_lowering=False,
        debug=False,
        enable_asserts=False,
        num_devices=n_cores,
    )
    h1 = nc.dram_tensor("h1", [n, d], BF16, kind="ExternalInput").ap()
    h2 = nc.dram_tensor("h2", [m, d], BF16, kind="ExternalInput").ap()
    maskf = nc.dram_tensor("maskf", [n, m], BF16, kind="ExternalInput").ap()
    G = nc.dram_tensor("G", [d, d], BF16, kind="ExternalInput").ap()
    WvT = nc.dram_tensor("WvT", [d, o], BF16, kind="ExternalInput").ap()
    ones = nc.dram_tensor("ones", [P, 1], BF16, kind="ExternalInput").ap()
    out = nc.dram_tensor("out", [n, o], F32, kind="ExternalOutput").ap()
    with tile.TileContext(nc) as tc:
        for _ in range(reps):
            emit_kernel(tc, h1, h2, maskf, G, WvT, ones, out, n, m, d, o, free)
    nc.compile()
    return nc


def _to_bf16(x_f32):
    """Fast vectorized fp32 -> bf16 with round-to-nearest-even."""
    x = np.ascontiguousarray(x_f32, dtype=np.float32)
    u = x.view(np.uint32)
    r = ((u >> np.uint32(16)) & np.uint32(1)) + np.uint32(0x7FFF)
    return ((u + r) >> np.uint32(16)).astype(np.uint16).view(ml_dtypes.bfloat16)


def prep_inputs(h1, h2, mask, Wq, Wk, Wv):
    """Host-side prep: fold Wq/Wk into G, transpose Wv, bf16-convert."""
    G = _to_bf16(Wq.astype(np.float32, copy=False).T @ Wk.astype(np.float32, copy=False))
    WvT = _to_bf16(np.ascontiguousarray(Wv.astype(np.float32, copy=False).T))
    h1b = _to_bf16(h1)
    h2b = _to_bf16(h2)
    # mask is 0/1 int32 -> bf16 0.0/1.0 via integer trick (0x3F80 == bf16 1.0)
    mb = (mask.astype(np.uint16) * np.uint16(0x3F80)).view(ml_dtypes.bfloat16)
    ones = np.ones((P, 1), dtype=ml_dtypes.bfloat16)
    return [
        {
            "h1": h1b[b],
            "h2": h2b[b],
            "maskf": mb[b],
            "G": G,
            "WvT": WvT,
            "ones": ones,
        }
        for b in range(B)
    ]


_NC_CACHE = {}


def get_nc():
    if "nc" not in _NC_CACHE:
        _NC_CACHE["nc"] = build_nc()
    return _NC_CACHE["nc"]


def run(in_maps, trace=False):
    return run_bass_kernel_spmd(get_nc(), in_maps, list(range(N_CORES)), trace=trace)


def kernel(h1, h2, mask, Wq, Wk, Wv):
    in_maps = prep_inputs(h1, h2, mask, Wq, Wk, Wv)
    res = run(in_maps)
    return np.stack([res.results[b]["out"] for b in range(B)], axis=0)
